# revision 1
# baseline (speedup 1.0000x reference)
"""Trainium2 Bass kernel for causal self-attention with RoPE.

Sharding: 8 cores = 2 batches x 4 head-groups (4 heads each).
Each core computes its batch's qkv projection for its heads, RoPE,
causal flash-attention, and a partial output projection; the host sums
the 4 partials per batch.

All matmuls run as fp32r (full-rate) except scores, whose operands are
bf16 (produced by the RoPE pass). Softmax uses no max-subtraction
(scores are O(5) bounded), and the denominator comes from an extra
ones-column in the PV stationary operand.

Schedule notes (v2):
- RoPE dims are interleaved (pair j at partitions 2j/2j+1), so the
  even/odd swap is a DVE stream_shuffle instead of SBUF-SBUF DMAs.
- qkv runs all-q/all-k/all-v per 512-token chunk with xt prefetch
  (bufs=16); q psums double-buffer across chunks (bufs=4) and the k/v
  evacuation lands inside the next chunk's q-matmul window.
- Softmax denominator: reciprocal -> partition_broadcast -> one DVE
  mul straight into u2_sb (no PE transposes, no DRAM round trip).
- proj(0) is held back to the tail so the last qb's denominator chain
  hides under it.
"""

import os

import numpy as np

NUM_HEADS = 16
B, T, C = 2, 2048, 1024
D = C // NUM_HEADS  # 64
HPC = 4             # heads per core
NCORES = 8

_CACHE = {}

LAST_EXEC_NS = None
LAST_RESULTS = None


def _build_body(nc, reps=1):
    import concourse.bass as bass
    import concourse.mybir as mybir
    import concourse.tile as tile
    from contextlib import ExitStack

    F32 = mybir.dt.float32
    F32R = mybir.dt.float32r
    BF16 = mybir.dt.bfloat16
    AF = mybir.ActivationFunctionType

    xT = nc.dram_tensor("xT", [C, T], BF16, kind="ExternalInput").ap()
    wT = nc.dram_tensor("wT", [C, 768], BF16, kind="ExternalInput").ap()
    projT = nc.dram_tensor("projT", [256, C], BF16, kind="ExternalInput").ap()
    CS = nc.dram_tensor("CS", [128, T], BF16, kind="ExternalInput").ap()
    SN = nc.dram_tensor("SN", [128, T], BF16, kind="ExternalInput").ap()
    maskc = nc.dram_tensor("maskc", [128, 128], BF16, kind="ExternalInput").ap()
    ident = nc.dram_tensor("ident", [128, 128], BF16, kind="ExternalInput").ap()
    out = nc.dram_tensor("out", [T, C], BF16, kind="ExternalOutput").ap()

    SWAP = [i ^ 1 for i in range(32)]

    with tile.TileContext(nc) as tc, ExitStack() as ctx:
        singles = ctx.enter_context(tc.tile_pool(name="singles", bufs=1))
        stream = ctx.enter_context(tc.tile_pool(name="stream", bufs=4))
        ptiles = ctx.enter_context(tc.tile_pool(name="ptiles", bufs=3))

        # weights first so the first qkv matmul's inputs arrive early
        w_sb = singles.tile([128, 8, 768], BF16)
        # dma_start costs ~1us of ISSUING-engine queue time, so the weight
        # loads are spread across gpsimd/scalar/vector; sync stays pure xt.
        # First ci split so the first q matmul's stationary arrives fast.
        nc.gpsimd.dma_start(out=w_sb[:, 0, 0:256], in_=wT[0:128, 0:256])
        nc.gpsimd.dma_start(out=w_sb[:, 0, 256:512], in_=wT[0:128, 256:512])
        nc.gpsimd.dma_start(out=w_sb[:, 0, 512:768], in_=wT[0:128, 512:768])
        for ci in range(2, 6):
            nc.scalar.dma_start(
                out=w_sb[:, ci, :], in_=wT[ci * 128:(ci + 1) * 128, :]
            )
        nc.gpsimd.dma_start(out=w_sb[:, 1, :], in_=wT[128:256, :])
        # ci6/ci7 ride the sync queue interleaved into ch0's xt stream
        cs_sb = singles.tile([128, T], BF16)
        sn_sb = singles.tile([128, T], BF16)
        nc.scalar.dma_start(out=cs_sb[:], in_=CS)
        nc.scalar.dma_start(out=sn_sb[:], in_=SN)
        mask_sb = singles.tile([128, 128], BF16)
        nc.scalar.dma_start(out=mask_sb[:], in_=maskc)
        id_sb = singles.tile([128, 128], BF16)
        nc.scalar.dma_start(out=id_sb[:], in_=ident)
        pj_sb = singles.tile([128, 2, C], BF16)

        # preload the Exp activation table while qkv runs
        dum_in = singles.tile([1, 2], F32)
        dum_out = singles.tile([1, 2], BF16)
        nc.vector.memset(dum_in[:], 0.0)
        nc.scalar.activation(dum_out[:], dum_in[:], AF.Exp)

        q_rot = singles.tile([128, 2, T], BF16)
        k_rot = singles.tile([128, 2, T], BF16)
        v_sb = singles.tile([128, 16, 65 * HPC], BF16)
        # per-(head, q-block) attention outputs at partitions 0-64
        # (row 64 = softmax denominator); block index r = h*4 + qb
        u_sb = singles.tile([65, 16, 512], F32)
        # normalized attention outputs, one tile per q-block so proj(qb)
        # only depends on chain(qb)
        u2q = [singles.tile([128, 2, 512], BF16, name=f"u2q{i}")
               for i in range(4)]

        # ones columns interleaved with v (col 64 of every 65-col head block)
        v_h = v_sb[:].rearrange("p t (h c) -> p t h c", c=65)
        nc.vector.memset(v_h[:, :, :, 64:65], 1.0)

        for rep in range(reps):
            # ---- Phase 1: QKV projection + RoPE ----
            with tc.tile_pool(name=f"qkvps{rep}", bufs=1, space="PSUM") as qkvps:
                for ch in range(4):
                    tok = slice(ch * 512, (ch + 1) * 512)
                    # per-ft psum tiles: q_ps[ft] = [feat 128, 512 toks],
                    # so each RoPE mul covers a whole tile in one DVE op
                    q_ps = [qkvps.tile([128, 512], F32, tag="qps", name="qps",
                                       bufs=4) for _ in range(2)]
                    k_ps = [qkvps.tile([128, 512], F32, tag="kps", name="kps",
                                       bufs=2) for _ in range(2)]
                    v_ps = [qkvps.tile([128, 512], F32, tag="vps", name="vps",
                                       bufs=2) for _ in range(2)]
                    xts = []
                    for ci in range(8):
                        xt = stream.tile([128, 512], BF16, tag="xt", name="xt",
                                         bufs=12)
                        if ch == 0 and ci == 0:
                            nc.sync.dma_start(out=xt[:, 0:256],
                                              in_=xT[0:128, 0:256])
                            nc.sync.dma_start(out=xt[:, 256:512],
                                              in_=xT[0:128, 256:512])
                        else:
                            nc.sync.dma_start(
                                out=xt[:], in_=xT[ci * 128:(ci + 1) * 128, tok]
                            )
                        if ch == 0 and ci in (2, 3):
                            wci = ci + 4
                            nc.sync.dma_start(
                                out=w_sb[:, wci, :],
                                in_=wT[wci * 128:(wci + 1) * 128, :],
                            )
                        xts.append(xt)

                    def mm_q(ci, sub, ft):
                        nc.tensor.matmul(
                            q_ps[ft][:, sub * 256:(sub + 1) * 256],
                            w_sb[:, ci, ft * 128:(ft + 1) * 128],
                            xts[ci][:, sub * 256:(sub + 1) * 256],
                            start=(ci == 0 and sub == 0),
                            stop=(ci == 7 and sub == 1),
                        )

                    def mm_k(ci, sub, ft):
                        nc.tensor.matmul(
                            k_ps[ft][:, sub * 256:(sub + 1) * 256],
                            w_sb[:, ci, 256 + ft * 128:256 + (ft + 1) * 128],
                            xts[ci][:, sub * 256:(sub + 1) * 256],
                            start=(ci == 0 and sub == 0),
                            stop=(ci == 7 and sub == 1),
                        )

                    def mm_v(ci, sub, ts):
                        nc.tensor.matmul(
                            v_ps[sub][:, ts * 256:(ts + 1) * 256],
                            xts[ci][:, sub * 256 + ts * 128:sub * 256 + (ts + 1) * 128],
                            w_sb[:, ci, 512:768],
                            start=(ci == 0 and ts == 0),
                            stop=(ci == 7 and ts == 1),
                        )

                    if ch == 0:
                        # cold chunk: interleave q/k/v per ci so the PE's
                        # consumption rate matches the xt DMA supply rate
                        for ci in range(8):
                            for sub in range(2):
                                for ft in range(2):
                                    mm_q(ci, sub, ft)
                                    mm_k(ci, sub, ft)
                                for ts in range(2):
                                    mm_v(ci, sub, ts)
                    else:
                        # warm chunks: all q, then k, then v — the k/v psum
                        # evacuation of chunk ch lands inside chunk ch+1's
                        # q-matmul window (q psums double-buffer via bufs=4)
                        for ci in range(8):
                            for sub in range(2):
                                for ft in range(2):
                                    mm_q(ci, sub, ft)
                        for ci in range(8):
                            for sub in range(2):
                                for ft in range(2):
                                    mm_k(ci, sub, ft)
                        for ci in range(8):
                            for sub in range(2):
                                for ts in range(2):
                                    mm_v(ci, sub, ts)

                    # RoPE (interleaved pairs): dest[2j] = ps[2j]*c -
                    # ps[2j+1]*s; dest[2j+1] = ps[2j+1]*c + ps[2j]*s.
                    # CS = [c,c] interleaved; SN = [+s,-s] interleaved; the
                    # pair swap is a DVE stream_shuffle (mask i^1).
                    # q on DVE, k on Pool so psum banks free in parallel.
                    # psum-reading muls all on DVE (GPSIMD cannot read
                    # PSUM on real hw), emitted first so banks free ASAP;
                    # the SBUF-only adds go to Pool
                    def rope_muls(src_ps, pre):
                        ts_ = []
                        for ft in range(2):
                            t1 = stream.tile([128, 512], BF16, tag=f"{pre}t1",
                                             name="t1", bufs=2)
                            t2 = stream.tile([128, 512], BF16, tag=f"{pre}t2",
                                             name="t2", bufs=2)
                            nc.vector.tensor_mul(t1[:], src_ps[ft][:],
                                                 cs_sb[:, tok])
                            nc.vector.tensor_mul(t2[:], src_ps[ft][:],
                                                 sn_sb[:, tok])
                            ts_.append((t1, t2))
                        return ts_

                    def rope_finish(ts_, dst, pre):
                        for ft in range(2):
                            t1, t2 = ts_[ft]
                            t2s = stream.tile([128, 512], BF16, tag=f"{pre}t2s",
                                              name="t2s", bufs=2)
                            nc.vector.stream_shuffle(t2s[:], t2[:], SWAP)
                            nc.gpsimd.tensor_add(dst[:, ft, tok], t1[:], t2s[:])

                    q_ts = rope_muls(q_ps, "q")
                    k_ts = rope_muls(k_ps, "k")
                    rope_finish(q_ts, q_rot, "q")
                    rope_finish(k_ts, k_rot, "k")
                    for sub in range(2):
                        for ts in range(2):
                            tokt = ch * 4 + sub * 2 + ts
                            src = v_ps[sub][:, ts * 256:(ts + 1) * 256]
                            nc.scalar.copy(
                                v_h[:, tokt, :, 0:64],
                                src.rearrange("p (h c) -> p h c", h=4),
                            )
                for hpi in range(2):
                    nc.sync.dma_start(
                        out=pj_sb[:, hpi, :], in_=projT[hpi * 128:(hpi + 1) * 128, :]
                    )

            # ---- Phase 2-4: causal attention + normalize + projection ----
            with tc.tile_pool(name=f"aps{rep}", bufs=1, space="PSUM") as aps:

                def emit_attn(hp, qb):
                    hA = 2 * hp
                    hB = 2 * hp + 1
                    nkt = 4 * qb + 4
                    uA = aps.tile([65, 512], F32, tag="uA", name="uA")
                    uB = aps.tile([65, 512], F32, tag="uB", name="uB")

                    def emit_scores(kt):
                        j = kt - 4 * qb
                        off = max(j, 0) * 128
                        ks = slice(kt * 128, (kt + 1) * 128)
                        qs = slice(qb * 512 + off, (qb + 1) * 512)
                        # both heads side by side in one 2-bank psum tile
                        sAB = aps.tile([128, 1024], F32, tag="sAB", name="sAB",
                                       bufs=2)
                        diag = j >= 0
                        nc.tensor.matmul(
                            sAB[:, off:512], k_rot[0:64, hp, ks],
                            q_rot[0:64, hp, qs], start=True, stop=not diag,
                        )
                        nc.tensor.matmul(
                            sAB[:, 512 + off:1024], k_rot[64:128, hp, ks],
                            q_rot[64:128, hp, qs], start=True, stop=not diag,
                        )
                        if diag:
                            # add the causal -inf wedge on the PE: I.T @ mask
                            nc.tensor.matmul(
                                sAB[:, off:off + 128], id_sb[:], mask_sb[:],
                                start=False, stop=True,
                            )
                            nc.tensor.matmul(
                                sAB[:, 512 + off:512 + off + 128], id_sb[:],
                                mask_sb[:], start=False, stop=True,
                            )
                        return sAB, off, kt

                    def emit_exp_pv(st):
                        sAB, off, kt = st
                        pAB = ptiles.tile([128, 1024], BF16, tag="pAB", name="pAB")
                        s_v = sAB[:].rearrange("p (b f) -> p b f", b=2)[:, :, off:512]
                        p_v = pAB[:].rearrange("p (b f) -> p b f", b=2)[:, :, off:512]
                        nc.scalar.activation(p_v, s_v, AF.Exp)
                        nc.tensor.matmul(
                            uA[0:65, off:512],
                            v_sb[:, kt, hA * 65:(hA + 1) * 65],
                            pAB[:, off:512],
                            start=(kt == 0), stop=(kt == nkt - 1),
                        )
                        nc.tensor.matmul(
                            uB[0:65, off:512],
                            v_sb[:, kt, hB * 65:(hB + 1) * 65],
                            pAB[:, 512 + off:1024],
                            start=(kt == 0), stop=(kt == nkt - 1),
                        )

                    prev = emit_scores(0)
                    for kt in range(1, nkt):
                        cur = emit_scores(kt)
                        emit_exp_pv(prev)
                        prev = cur
                    emit_exp_pv(prev)

                    for u_ps, h in ((uA, hA), (uB, hB)):
                        r = h * 4 + qb
                        nc.vector.tensor_copy(u_sb[:, r, :], u_ps[0:65, :])

                def emit_chain(qb, hp):
                    # 1/l broadcast across the 64 head dims, one mul into
                    # u2q[qb]: no PE transposes, no DRAM round trip.
                    # Split per hp so proj's first (plane-0) psum matmul only
                    # depends on the hp0 chain.
                    for hh in range(2):
                        h = 2 * hp + hh
                        r = h * 4 + qb
                        lrow = stream.tile([1, 512], F32, tag="lrow",
                                           name="lrow", bufs=2)
                        nc.vector.reciprocal(lrow[:], u_sb[64:65, r, :])
                        lb = stream.tile([64, 512], F32, tag="lb", name="lb")
                        nc.gpsimd.partition_broadcast(lb[:], lrow[0:1, :],
                                                      channels=64)
                        nc.vector.tensor_mul(
                            u2q[qb][hh * 64:(hh + 1) * 64, hp, :],
                            u_sb[0:64, r, :], lb[:],
                        )

                def emit_proj(qb, half=None, tail=False, pool=None,
                              ppbufs=2):
                    # tail projections evacuate psum on ACT+DVE and use a
                    # fresh full-width psum pool (no bank ping-pong)
                    pool = pool or aps
                    ms_range = (range(4) if half is None else
                                range(2 * half, 2 * half + 2))
                    for mq in ms_range:
                        m = 4 * qb + mq
                        ms = slice(m * 128, (m + 1) * 128)
                        mq_s = slice(mq * 128, (mq + 1) * 128)
                        ob = stream.tile([128, 1024], BF16, tag="ob", name="ob", bufs=6)
                        for nh in range(2):
                            pp = pool.tile([128, 512], F32, tag="pp", name="pp",
                                           bufs=ppbufs)
                            nc.tensor.matmul(
                                pp[:],
                                u2q[qb][:, 0, mq_s],
                                pj_sb[:, 0, nh * 512:(nh + 1) * 512],
                                start=True, stop=False,
                            )
                            nc.tensor.matmul(
                                pp[:],
                                u2q[qb][:, 1, mq_s],
                                pj_sb[:, 1, nh * 512:(nh + 1) * 512],
                                start=False, stop=True,
                            )
                            ce = ((nc.scalar.copy if nh == 0 else
                                   nc.vector.tensor_copy) if tail else
                                  nc.vector.tensor_copy)
                            ce(ob[:, nh * 512:(nh + 1) * 512], pp[:])
                        (nc.sync if m % 2 == 0 else nc.gpsimd).dma_start(
                            out=out[ms, :], in_=ob[:]
                        )

                # projections are spread into the (ACT-bound) attention
                # windows as soon as their chain is done; the last ones
                # cover the latency of qb3's denominator chain
                for qb in range(4):
                    emit_attn(0, qb)
                    emit_chain(qb, 0)
                    if qb == 2:
                        emit_proj(0, half=1)
                    elif qb == 3:
                        emit_proj(1, half=1)
                    emit_attn(1, qb)
                    emit_chain(qb, 1)
                    if qb == 1:
                        emit_proj(0, half=0)
                    elif qb == 2:
                        emit_proj(1, half=0)
                    elif qb == 3:
                        emit_proj(2, half=0)

            with tc.tile_pool(name=f"tailps{rep}", bufs=1,
                              space="PSUM") as tailps:
                emit_proj(2, half=1, tail=True, pool=tailps, ppbufs=8)
                emit_proj(3, tail=True, pool=tailps, ppbufs=8)

    return nc


def _get_nc(reps=1):
    key = f"nc{reps}"
    if key not in _CACHE:
        import concourse.bacc as bacc

        nc = bacc.Bacc("TRN2", target_bir_lowering=False, debug=False)
        _build_body(nc, reps=reps)
        nc.compile()
        _CACHE[key] = nc
    return _CACHE[key]


def _prep_in_maps(x, freqs_cos, freqs_sin, qkv_w, proj_w):
    x = np.asarray(x, dtype=np.float32)
    cos = np.asarray(freqs_cos, dtype=np.float32)
    sin = np.asarray(freqs_sin, dtype=np.float32)
    qkv_w = np.asarray(qkv_w, dtype=np.float32)
    proj_w = np.asarray(proj_w, dtype=np.float32)

    import ml_dtypes
    BF = ml_dtypes.bfloat16
    sq = np.float32((1.0 / np.sqrt(D)) ** 0.5)
    cosT = np.ascontiguousarray(cos.T) * sq  # (32, T)
    sinT = np.ascontiguousarray(sin.T) * sq
    # interleaved pair layout: partition 2j/2j+1 = pair j
    ci64 = np.repeat(cosT, 2, axis=0)        # (64, T)
    si64 = np.repeat(sinT, 2, axis=0)
    si64[1::2] *= -1.0                       # SN = [+s, -s] interleaved
    CS = np.tile(ci64, (2, 1)).astype(BF)
    SN = np.tile(si64, (2, 1)).astype(BF)
    f = np.arange(128)
    maskc = np.where(f[None, :] >= f[:, None], 0.0, -1e30).astype(BF)
    ident = np.eye(128, dtype=np.float32).astype(BF)

    in_maps = []
    for core in range(NCORES):
        b = core // 4
        g = core % 4
        heads = [4 * g + j for j in range(HPC)]
        q_rows = np.concatenate([h * D + np.arange(D) for h in heads])
        k_rows = np.concatenate([C + h * D + np.arange(D) for h in heads])
        v_rows = np.concatenate([2 * C + h * D + np.arange(D) for h in heads])
        wTc = np.ascontiguousarray(
            np.concatenate(
                [qkv_w[q_rows, :], qkv_w[k_rows, :], qkv_w[v_rows, :]], axis=0
            ).T
        ).astype(BF)  # (1024, 768)
        vcols = np.concatenate([h * D + np.arange(D) for h in heads])
        projTc = np.ascontiguousarray(proj_w[:, vcols].T).astype(BF)
        xTc = np.ascontiguousarray(x[b].T).astype(BF)  # (1024, 2048)
        in_maps.append(
            {
                "xT": xTc,
                "wT": wTc,
                "projT": projTc,
                "CS": CS,
                "SN": SN,
                "maskc": maskc,
                "ident": ident,
            }
        )
    return in_maps


def _get_runner(reps=1):
    """Build (once) a jitted 8-core shard_map executable mirroring
    bass2jax.run_bass_via_pjrt, without donation so it can be re-run for
    timing with device-resident inputs."""
    rkey = f"runner{reps}"
    if rkey in _CACHE:
        return _CACHE[rkey]
    import jax
    import concourse.mybir as mybir
    from concourse import bass2jax
    from jax.experimental.shard_map import shard_map
    from jax.sharding import Mesh, PartitionSpec

    nc = _get_nc(reps)
    bass2jax.install_neuronx_cc_hook()

    in_names = []
    out_names = []
    out_avals = []
    zero_outs = []
    pname = nc.partition_id_tensor.name if nc.partition_id_tensor else None
    for alloc in nc.m.functions[0].allocations:
        if not isinstance(alloc, mybir.MemoryLocationSet):
            continue
        name = alloc.memorylocations[0].name
        if alloc.kind == "ExternalInput":
            if name != pname:
                in_names.append(name)
        elif alloc.kind == "ExternalOutput":
            shape = tuple(alloc.tensor_shape)
            dtype = mybir.dt.np(alloc.dtype)
            out_names.append(name)
            out_avals.append(jax.core.ShapedArray(shape, dtype))
            zero_outs.append(np.zeros(shape, dtype))
    n_params = len(in_names)
    all_names = list(in_names) + list(out_names)
    if pname is not None:
        all_names.append(pname)

    def _body(*args):
        operands = list(args)
        if pname is not None:
            operands.append(bass2jax.partition_id_tensor())
        outs = bass2jax._bass_exec_p.bind(
            *operands,
            out_avals=tuple(out_avals),
            in_names=tuple(all_names),
            out_names=tuple(out_names),
            lowering_input_output_aliases=(),
            sim_require_finite=True,
            sim_require_nnan=True,
            nc=nc,
        )
        return tuple(outs)

    devices = jax.devices()[:NCORES]
    mesh = Mesh(np.asarray(devices), ("core",))
    nin = n_params + len(out_names)
    sharded_body = shard_map(
        _body,
        mesh=mesh,
        in_specs=(PartitionSpec("core"),) * nin,
        out_specs=(PartitionSpec("core"),) * len(out_names),
        check_rep=False,
    )
    sharded = jax.jit(sharded_body, keep_unused=True)
    _CACHE[rkey] = (sharded, in_names, out_names, zero_outs, mesh)
    return _CACHE[rkey]


def kernel(x, freqs_cos, freqs_sin, qkv_w, proj_w):
    import jax
    from jax.sharding import NamedSharding, PartitionSpec

    global LAST_EXEC_NS, LAST_RESULTS
    sharded, in_names, out_names, zero_outs, mesh = _get_runner()
    in_maps = _prep_in_maps(x, freqs_cos, freqs_sin, qkv_w, proj_w)

    concat_in = [
        np.concatenate([in_maps[c][n] for c in range(NCORES)], axis=0)
        for n in in_names
    ]
    concat_zero = [
        np.zeros((NCORES * z.shape[0], *z.shape[1:]), z.dtype) for z in zero_outs
    ]
    sharding = NamedSharding(mesh, PartitionSpec("core"))
    dev_args = [jax.device_put(a, sharding) for a in concat_in + concat_zero]

    out_arrs = sharded(*dev_args)
    jax.block_until_ready(out_arrs)

    iters = int(os.environ.get("KERNEL_TIME_ITERS", "0"))
    if iters > 0:
        import time

        # Differential timing: one NEFF with the body repeated 8x vs 1x;
        # paired alternating rounds cancel dispatch overhead and drift.
        sharded8 = _get_runner(reps=8)[0]
        jax.block_until_ready(sharded8(*dev_args))

        def one_round(fn):
            t0 = time.monotonic()
            for _ in range(iters):
                r = fn(*dev_args)
            jax.block_until_ready(r)
            return (time.monotonic() - t0) / iters

        diffs = []
        for _ in range(6):
            t1 = one_round(sharded)
            t8 = one_round(sharded8)
            diffs.append((t8 - t1) / 7 * 1e9)
        diffs.sort()
        LAST_EXEC_NS = diffs[len(diffs) // 2]
        _CACHE["exec_ns_min"] = diffs[0]

    out = np.asarray(out_arrs[out_names.index("out")]).astype(np.float32)
    out = out.reshape(NCORES, T, C)
    return np.stack(
        [
            out[0] + out[1] + out[2] + out[3],
            out[4] + out[5] + out[6] + out[7],
        ],
        axis=0,
    )

